# revision 1
# baseline (speedup 1.0000x reference)
"""HeteroLinear (per-token expert linear) on 8 TRN2 NeuronCores.

Strategy: expert-parallel. The reference computes all 8 GEMMs on every
token and masks (8x redundant compute). Here the host routes tokens to
their expert: tokens of type t go to core t, padded to a static
capacity C. Each core then runs ONE dense [C,1024]@[1024,1024] GEMM in
bf16 — the algorithmic minimum of compute — with the bias add fused
into the PSUM eviction. The host un-permutes the results.

All layout work (permute, transpose, f32->bf16 cast) happens on the
host so the device kernel is a pure weights-stationary matmul:
  inputs per core:  xT [IN, C] bf16 (tokens transposed), w [IN, OUT]
                    bf16, b [128, OUT/128] f32
  output per core:  outT [OUT, C] bf16  (= (x @ W + b)^T)

A demand-ordered startup-DMA / dual-queue / DVE-eviction rework sims
4 us faster (TimelineSim 68.6 vs 72.7 us) but measured equal-or-
slightly-WORSE per-exec through the axon tunnel in interleaved A/B, so
this simpler schedule is kept (see exp_ab4.py and memory notes).
"""

import numpy as np
import ml_dtypes

import concourse.bass as bass
import concourse.mybir as mybir
import concourse.tile as tile
from concourse import bacc
from concourse.bass import ts
from concourse.bass_utils import run_bass_kernel_spmd

N_CORES = 8
T = 8           # experts
IN = 1024
OUT = 1024
P = 128
KC = IN // P    # contraction chunks
MC = OUT // P   # output-row chunks
C_DEFAULT = 2176  # token capacity per core; multiple of 128

_BF16 = ml_dtypes.bfloat16

_nc_cache: dict[int, object] = {}


def _token_chunks(C):
    chunks = []
    off = 0
    while off < C:
        w = min(512, C - off)
        chunks.append((off, w))
        off += w
    return chunks


def _build(C):
    """Build + compile the per-core GEMM program (same on all cores)."""
    nc = bacc.Bacc(
        "TRN2", target_bir_lowering=False, debug=False, num_devices=N_CORES
    )
    xT = nc.dram_tensor("xt", [IN, C], mybir.dt.bfloat16, kind="ExternalInput").ap()
    w = nc.dram_tensor("w", [IN, OUT], mybir.dt.bfloat16, kind="ExternalInput").ap()
    bb = nc.dram_tensor("b", [P, MC], mybir.dt.float32, kind="ExternalInput").ap()
    outT = nc.dram_tensor(
        "outt", [OUT, C], mybir.dt.bfloat16, kind="ExternalOutput"
    ).ap()

    chunks = _token_chunks(C)

    outT_re = outT.rearrange("(mc p) c -> p mc c", p=P)
    xT_re = xT.rearrange("(kc p) c -> p kc c", p=P)

    with tile.TileContext(nc) as tc:
        with (
            tc.tile_pool(name="wpool", bufs=1) as wpool,
            tc.tile_pool(name="xpool", bufs=1) as xpool,
            tc.tile_pool(name="bpool", bufs=1) as bpool,
            tc.tile_pool(name="opool", bufs=3) as opool,
            tc.tile_pool(name="psum", bufs=8, space="PSUM") as pspool,
        ):
            b_sb = bpool.tile([P, MC], mybir.dt.float32)
            w_sb = wpool.tile([P, KC, OUT], mybir.dt.bfloat16)
            w_re = w.rearrange("(kc p) o -> p kc o", p=P)
            x_sb = xpool.tile([P, KC, C], mybir.dt.bfloat16)
            W_SPLIT = 256  # first two column blocks
            nc.sync.dma_start(w_sb[:, :, :P], w_re[:, :, :P])
            nc.sync.dma_start(w_sb[:, :, P:W_SPLIT], w_re[:, :, P:W_SPLIT])
            wd0 = chunks[0][1]
            for j in range(0, KC, 2):
                nc.sync.dma_start(
                    x_sb[:, j : j + 2, :wd0], xT_re[:, j : j + 2, :wd0]
                )
            nc.sync.dma_start(w_sb[:, :, W_SPLIT:], w_re[:, :, W_SPLIT:])
            nc.sync.dma_start(b_sb[:], bb[:])
            for ci, (off, wd) in enumerate(chunks):
                if ci == 0:
                    continue
                nc.sync.dma_start(
                    x_sb[:, :, off : off + wd], xT_re[:, :, off : off + wd]
                )

            for ci, (off, wd) in enumerate(chunks):
                ost = opool.tile([P, MC, 512], mybir.dt.bfloat16, tag="ost")
                for m0 in range(0, MC, 2):
                    pts = [
                        pspool.tile(
                            [P, 512], mybir.dt.float32, tag="ps",
                            name=f"ps_{ci}_{m0}_{j}",
                        )
                        for j in range(2)
                    ]
                    for k in range(KC):
                        for j in range(2):
                            nc.tensor.matmul(
                                pts[j][:, :wd],
                                w_sb[:, k, ts(m0 + j, P)],
                                x_sb[:, k, off : off + wd],
                                start=(k == 0),
                                stop=(k == KC - 1),
                            )
                    for j in range(2):
                        nc.scalar.activation(
                            ost[:, m0 + j, :wd],
                            pts[j][:, :wd],
                            mybir.ActivationFunctionType.Identity,
                            bias=b_sb[:, m0 + j : m0 + j + 1],
                        )
                if ci == len(chunks) - 1:
                    half = MC // 2
                    nc.sync.dma_start(
                        outT_re[:, :half, off : off + wd], ost[:, :half, :wd]
                    )
                    nc.sync.dma_start(
                        outT_re[:, half:, off : off + wd], ost[:, half:, :wd]
                    )
                else:
                    nc.sync.dma_start(
                        outT_re[:, :, off : off + wd], ost[:, :, :wd]
                    )

    nc.compile()
    return nc


def _get_nc(C):
    if C not in _nc_cache:
        _nc_cache[C] = _build(C)
    return _nc_cache[C]


def _route(x, types):
    """Group token indices by expert type."""
    x = np.asarray(x)
    types = np.asarray(types)
    B, S, _ = x.shape
    x_flat = np.ascontiguousarray(x.reshape(B * S, IN))
    t_flat = types.reshape(B * S).astype(np.int64)
    order = np.argsort(t_flat, kind="stable")
    counts = np.bincount(t_flat, minlength=T)
    idx_lists = []
    off = 0
    for t in range(T):
        idx_lists.append(order[off : off + counts[t]])
        off += counts[t]
    return x_flat, idx_lists, counts


def _make_in_maps(x_flat, idx_lists, W, b, C):
    W = np.asarray(W)
    b = np.asarray(b)
    in_maps = []
    for t in range(T):
        idx = idx_lists[t]
        n = len(idx)
        xTt = np.zeros((IN, C), dtype=_BF16)
        if n:
            xTt[:, :n] = x_flat[idx].astype(_BF16).T
        in_maps.append(
            {
                "xt": xTt,
                "w": W[t].astype(_BF16),
                "b": np.ascontiguousarray(
                    b[t].astype(np.float32).reshape(MC, P).T
                ),
            }
        )
    return in_maps


def kernel(x, types, W, b):
    x = np.asarray(x)
    B, S, _ = x.shape
    x_flat, idx_lists, counts = _route(x, types)
    # pad capacity up to C_DEFAULT (the canonical-input worst case) so
    # repeated calls and the timing harness share one compiled program;
    # the extra tokens are <5% of the GEMM and compile is ~100s
    C = max(C_DEFAULT, (int(counts.max()) + P - 1) // P * P)
    nc = _get_nc(C)
    in_maps = _make_in_maps(x_flat, idx_lists, W, b, C)
    res = run_bass_kernel_spmd(nc, in_maps, list(range(N_CORES)), trace=False)
    out_flat = np.empty((B * S, OUT), dtype=np.float32)
    for t in range(T):
        idx = idx_lists[t]
        if len(idx):
            out_flat[idx] = res.results[t]["outt"][:, : len(idx)].T.astype(
                np.float32
            )
    return out_flat.reshape(B, S, OUT)

